# revision 3
# baseline (speedup 1.0000x reference)
"""Trainium2 Bass kernel for GCN ExitBlock: out = (adj @ (x @ gc_W) + gc_b) @ fc_W + fc_b.

Strategy (8 NeuronCores, SPMD, no collectives):
  - Row-shard the output: core c computes rows [1500c, 1500(c+1)).
  - Host pre-transposes adj so each core receives adjT_c = adj[rows_c, :].T
    ([12000, 1500], contiguous) -- puts the contraction dim on SBUF partitions
    with fully contiguous 768 KB slab DMAs.
  - On device, per k-tile of 128: support_tile = (xT_slice).T @ gc_W on the PE,
    then hT += support_tile.T @ adjT_slab accumulated in PSUM (f32r matmuls:
    1 cyc/row vs 4 for f32, tf32-class precision, fp32 accumulate).
  - Epilogue: hT += gc_b (DVE), append a ones-row, outT = [fc_W; fc_b].T @ hs
    on the PE, one 96 KB store.
  - Host gathers the 8 outT blocks ([16, 1500]) and transposes to [12000, 16].

The kernel is HBM-bound: 72 MB of adj per core @ ~358 GB/s => ~201 us roofline.
"""
import sys

sys.path.insert(0, "/opt/trn_rl_repo")

import numpy as np

N, NHID, NCLASS, NCORES = 12000, 32, 16, 8
R = N // NCORES            # 1500 rows per core
KP = 128                   # contraction tile (SBUF partitions)
K_TILES = [(k0, min(KP, N - k0)) for k0 in range(0, N, KP)]   # 93x128 + 1x96
R_SPLITS = [(0, 512), (512, 512), (1024, R - 1024)]           # matmul N<=512
X_CHUNKS = 4               # xT preload split so the k-loop starts early

_cached = {}


def _build_nc():
    import concourse.bacc as bacc
    import concourse.mybir as mybir
    from concourse import tile

    f32 = mybir.dt.float32
    f32r = mybir.dt.float32r

    nc = bacc.Bacc()
    xT_d = nc.declare_dram_parameter("xT", [NHID, N], f32, isOutput=False)
    adjT_d = nc.declare_dram_parameter("adjT", [N, R], f32r, isOutput=False)
    gcW_d = nc.declare_dram_parameter("gcW", [NHID, NHID], f32, isOutput=False)
    gcb_d = nc.declare_dram_parameter("gcb", [NHID, 1], f32, isOutput=False)
    fcWb_d = nc.declare_dram_parameter("fcWb", [NHID + 1, NCLASS], f32, isOutput=False)
    outT_d = nc.declare_dram_parameter("outT", [NCLASS, R], f32, isOutput=True)

    with tile.TileContext(nc) as tc:
        with (
            tc.tile_pool(name="cst", bufs=1) as cst,
            tc.tile_pool(name="adj", bufs=3) as adjp,
            tc.tile_pool(name="sup", bufs=3) as supp,
            tc.tile_pool(name="ps_s", bufs=2, space="PSUM") as ps_s,
            tc.tile_pool(name="ps_h", bufs=1, space="PSUM") as ps_h,
            tc.tile_pool(name="ps_o", bufs=2, space="PSUM") as ps_o,
        ):
            # ---- constant preloads ----
            xT_sb = cst.tile([NHID, N], f32)
            cw = N // X_CHUNKS
            for c in range(X_CHUNKS):
                nc.sync.dma_start(xT_sb[:, c * cw:(c + 1) * cw],
                                  xT_d[:, c * cw:(c + 1) * cw])
            gcW_sb = cst.tile([NHID, NHID], f32)
            nc.sync.dma_start(gcW_sb[:], gcW_d[:])
            gcb_sb = cst.tile([NHID, 1], f32)
            nc.sync.dma_start(gcb_sb[:], gcb_d[:])
            fcWb_sb = cst.tile([NHID + 1, NCLASS], f32)
            nc.sync.dma_start(fcWb_sb[:], fcWb_d[:])

            # hs = [hT + gc_b ; ones] in f32r, feeds the classifier matmul
            hs_sb = cst.tile([NHID + 1, R], f32)
            nc.vector.memset(hs_sb[NHID:NHID + 1, :], 1.0)

            hps = [ps_h.tile([NHID, n], f32, name=f"hps{j}", tag=f"hps{j}")
                   for j, (_, n) in enumerate(R_SPLITS)]

            # ---- main streaming loop over contraction tiles ----
            tlast = len(K_TILES) - 1
            for t, (k0, P) in enumerate(K_TILES):
                a_sb = adjp.tile([KP, R], f32r, name="a_sb", tag="a")
                nc.sync.dma_start(a_sb[:P, :], adjT_d[k0:k0 + P, :])

                s_ps = ps_s.tile([KP, NHID], f32, name="s_ps", tag="s_ps")
                nc.tensor.matmul(s_ps[:P, :], xT_sb[:, k0:k0 + P], gcW_sb[:],
                                 start=True, stop=True)
                s_sb = supp.tile([KP, NHID], f32r, name="s_sb", tag="s_sb")
                nc.vector.tensor_copy(s_sb[:P, :], s_ps[:P, :])

                for j, (c0, cn) in enumerate(R_SPLITS):
                    nc.tensor.matmul(hps[j][:, :], s_sb[:P, :],
                                     a_sb[:P, c0:c0 + cn],
                                     start=(t == 0), stop=(t == tlast))

            # ---- epilogue: bias, classifier, store ----
            o_sb = cst.tile([NCLASS, R], f32)
            for j, (c0, cn) in enumerate(R_SPLITS):
                nc.vector.tensor_scalar_add(hs_sb[0:NHID, c0:c0 + cn],
                                            hps[j][:, :], gcb_sb[:, 0:1])
                o_ps = ps_o.tile([NCLASS, 512], f32, name="o_ps", tag="o_ps")
                nc.tensor.matmul(o_ps[:, :cn], fcWb_sb[:], hs_sb[:, c0:c0 + cn],
                                 start=True, stop=True)
                nc.vector.tensor_copy(o_sb[:, c0:c0 + cn], o_ps[:, :cn])
            nc.sync.dma_start(outT_d[:], o_sb[:])

    nc.finalize()
    return nc


def _get_nc():
    if "nc" not in _cached:
        _cached["nc"] = _build_nc()
    return _cached["nc"]


def _prep_in_maps(x, adj, gc_W, gc_b, fc_W, fc_b):
    f = np.float32
    xT = np.ascontiguousarray(np.asarray(x, dtype=f).T)                  # [32, N]
    adj = np.asarray(adj, dtype=f)
    # [8, 12000, 1500]: block c = adj[rows_c, :].T, contiguous
    adjT = np.ascontiguousarray(
        adj.reshape(NCORES, R, N).transpose(0, 2, 1))
    gcW = np.ascontiguousarray(np.asarray(gc_W, dtype=f))
    gcb = np.ascontiguousarray(np.asarray(gc_b, dtype=f).reshape(NHID, 1))
    fcWb = np.ascontiguousarray(
        np.concatenate([np.asarray(fc_W, dtype=f),
                        np.asarray(fc_b, dtype=f).reshape(1, NCLASS)], axis=0))
    return [{"xT": xT, "adjT": adjT[c], "gcW": gcW, "gcb": gcb, "fcWb": fcWb}
            for c in range(NCORES)]


def run_traced(x, adj, gc_W, gc_b, fc_W, fc_b, trace=False, **kw):
    """Run on the 8 NeuronCores; returns (out [N, NCLASS] f32, BassKernelResults)."""
    from concourse.bass_utils import run_bass_kernel_spmd

    nc = _get_nc()
    in_maps = _prep_in_maps(x, adj, gc_W, gc_b, fc_W, fc_b)
    res = run_bass_kernel_spmd(nc, in_maps, list(range(NCORES)), trace=trace, **kw)
    outT = np.concatenate([res.results[c]["outT"] for c in range(NCORES)], axis=1)
    out = np.ascontiguousarray(outT.T).astype(np.float32, copy=False)
    return out, res


def kernel(x, adj, gc_W, gc_b, fc_W, fc_b):
    out, _ = run_traced(x, adj, gc_W, gc_b, fc_W, fc_b, trace=False)
    return out


# revision 6
# speedup vs baseline: 1.4452x; 1.4452x over previous
"""Trainium2 Bass kernel for GCN ExitBlock: out = (adj @ (x @ gc_W) + gc_b) @ fc_W + fc_b.

Strategy (8 NeuronCores, SPMD, no collectives):
  - Reassociate: out = ((adj @ x) @ gc_W + gc_b) @ fc_W + fc_b.  The big
    streaming matmul g = adj @ x then uses x in its NATURAL [k, 32] layout as
    the PE's stationary operand -- no transposes and no per-tile prep work.
  - Row-shard the output: core c computes rows [1500c, 1500(c+1)).
  - Host pre-transposes adj: core c receives adjT_c = adj[rows_c, :].T
    ([12032, 1500] zero-padded, contiguous) so the contraction dim lands on
    SBUF partitions.  k-tiles are batched into multi-tile slabs (p-interleaved:
    slab row p holds k = k0 + G*p + j) keeping per-partition DMA contiguous at
    G*6000 B; x is pre-permuted on the host to match.
  - Per sub-tile: gT[32,1500] += x_tile.T @ adjT_slab in f32r (1-pass matmuls,
    tf32-class precision, fp32 PSUM accumulate).
  - Epilogue: hT = gc_W.T @ gT; outT = fc_W.T @ hT + (fc_W.T gc_b + fc_b);
    biases folded into a single [16,1] vector via a tiny matmul.
  - Host gathers the 8 outT blocks ([16, 1500]) and transposes to [12000, 16].

HBM-bound: 72 MB of adj per core @ ~358 GB/s => ~202 us roofline.
"""
import sys

sys.path.insert(0, "/opt/trn_rl_repo")

import numpy as np

N, NHID, NCLASS, NCORES = 12000, 32, 16, 8
R = N // NCORES            # 1500 rows per core
KP = 128                   # partitions per sub-tile
NT = 94                    # sub-tiles (12032 padded k rows)
NPAD = NT * KP             # 12032
GROUPS = [4] * 23 + [2]    # sub-tiles per DMA slab (3 MB / 1.5 MB transfers)
assert sum(GROUPS) == NT
R_SPLITS = [(0, 512), (512, 512), (1024, R - 1024)]           # matmul N<=512

_cached = {}


def _build_nc():
    import concourse.bacc as bacc
    import concourse.mybir as mybir
    from concourse import tile

    f32 = mybir.dt.float32
    f32r = mybir.dt.float32r

    nc = bacc.Bacc()
    xP_d = nc.declare_dram_parameter("xP", [KP, NT * NHID], f32r, isOutput=False)
    adjT_d = nc.declare_dram_parameter("adjT", [NPAD, R], f32r, isOutput=False)
    gcW_d = nc.declare_dram_parameter("gcW", [NHID, NHID], f32r, isOutput=False)
    fcW_d = nc.declare_dram_parameter("fcW", [NHID, NCLASS], f32r, isOutput=False)
    # cvec = [gc_b; 1] followed by [fc_W; fc_b] rows -> bias vector via matmul
    fcWb_d = nc.declare_dram_parameter("fcWb", [NHID + 1, NCLASS], f32, isOutput=False)
    gcb1_d = nc.declare_dram_parameter("gcb1", [NHID + 1, 1], f32, isOutput=False)
    outT_d = nc.declare_dram_parameter("outT", [NCLASS, R], f32, isOutput=True)

    with tile.TileContext(nc) as tc:
        with (
            tc.tile_pool(name="cst", bufs=1) as cst,
            tc.tile_pool(name="adj", bufs=3) as adjp,
            tc.tile_pool(name="ps_g", bufs=1, space="PSUM") as ps_g,
            tc.tile_pool(name="ps_e", bufs=1, space="PSUM") as ps_e,
        ):
            # ---- constant preloads ----
            x_sb = cst.tile([KP, NT, NHID], f32r)
            nc.sync.dma_start(x_sb[:], xP_d.rearrange("p (t j) -> p t j", j=NHID))
            gcW_sb = cst.tile([NHID, NHID], f32r)
            nc.sync.dma_start(gcW_sb[:], gcW_d[:])
            fcW_sb = cst.tile([NHID, NCLASS], f32r)
            nc.sync.dma_start(fcW_sb[:], fcW_d[:])
            fcWb_sb = cst.tile([NHID + 1, NCLASS], f32)
            nc.sync.dma_start(fcWb_sb[:], fcWb_d[:])
            gcb1_sb = cst.tile([NHID + 1, 1], f32)
            nc.sync.dma_start(gcb1_sb[:], gcb1_d[:])

            gps = [ps_g.tile([NHID, n], f32, name=f"gps{j}", tag=f"gps{j}")
                   for j, (_, n) in enumerate(R_SPLITS)]

            # bias vector c = fcWb.T @ [gc_b; 1] = fc_W.T gc_b + fc_b  [16, 1]
            c_ps = ps_e.tile([NCLASS, 1], f32, name="c_ps", tag="c_ps")
            nc.tensor.matmul(c_ps[:], fcWb_sb[:], gcb1_sb[:], start=True, stop=True)
            c_sb = cst.tile([NCLASS, 1], f32)
            nc.vector.tensor_copy(c_sb[:], c_ps[:])

            # ---- main streaming loop: gT += x_tile.T @ adjT_slab ----
            s = 0          # global sub-tile index
            k0 = 0
            ngroups = len(GROUPS)
            for g, G in enumerate(GROUPS):
                a_sb = adjp.tile([KP, 4, R], f32r, name="a_sb", tag="a")
                eng = nc.sync if (g % 2 == 0) else nc.scalar
                eng.dma_start(
                    a_sb[:, :G, :],
                    adjT_d[k0:k0 + KP * G, :].rearrange("(p j) r -> p j r", j=G))
                for j in range(G):
                    st = (s == 0)
                    sp = (s == NT - 1)
                    for q, (c0, cn) in enumerate(R_SPLITS):
                        nc.tensor.matmul(gps[q][:, :], x_sb[:, s, :],
                                         a_sb[:, j, c0:c0 + cn],
                                         start=st, stop=sp)
                    s += 1
                k0 += KP * G

            # ---- epilogue: hT = gcW.T @ gT;  outT = fcW.T @ hT + c ----
            g_sb = cst.tile([NHID, R], f32r)
            hT_sb = cst.tile([NHID, R], f32r)
            o_sb = cst.tile([NCLASS, R], f32)
            for q, (c0, cn) in enumerate(R_SPLITS):
                nc.vector.tensor_copy(g_sb[:, c0:c0 + cn], gps[q][:, :])
                h_ps = ps_e.tile([NHID, 512], f32, name="h_ps", tag="h_ps")
                nc.tensor.matmul(h_ps[:, :cn], gcW_sb[:], g_sb[:, c0:c0 + cn],
                                 start=True, stop=True)
                nc.vector.tensor_copy(hT_sb[:, c0:c0 + cn], h_ps[:, :cn])
                o_ps = ps_e.tile([NCLASS, 512], f32, name="o_ps", tag="o_ps")
                nc.tensor.matmul(o_ps[:, :cn], fcW_sb[:], hT_sb[:, c0:c0 + cn],
                                 start=True, stop=True)
                nc.vector.tensor_scalar_add(o_sb[:, c0:c0 + cn], o_ps[:, :cn],
                                            c_sb[:, 0:1])
            nc.sync.dma_start(outT_d[:], o_sb[:])

    nc.finalize()
    return nc


def _get_nc():
    if "nc" not in _cached:
        _cached["nc"] = _build_nc()
    return _cached["nc"]


def _prep_in_maps(x, adj, gc_W, gc_b, fc_W, fc_b):
    f = np.float32
    x = np.asarray(x, dtype=f)
    adj = np.asarray(adj, dtype=f)

    # x permuted to match the slab interleave: xP[p, s*NHID:(s+1)*NHID] is the
    # stationary operand of sub-tile s, whose partition p holds k = k0+G*p+j.
    xpad = np.zeros((NPAD, NHID), dtype=f)
    xpad[:N] = x
    xP = np.empty((KP, NT, NHID), dtype=f)
    s = 0
    k0 = 0
    for G in GROUPS:
        blk = xpad[k0:k0 + KP * G].reshape(KP, G, NHID)
        for j in range(G):
            xP[:, s, :] = blk[:, j, :]
            s += 1
        k0 += KP * G
    xP = np.ascontiguousarray(xP.reshape(KP, NT * NHID))

    # adjT blocks: [12032, 1500] per core (32 zero-padded k rows)
    adjT = np.zeros((NCORES, NPAD, R), dtype=f)
    adjT[:, :N, :] = adj.reshape(NCORES, R, N).transpose(0, 2, 1)

    gcW = np.ascontiguousarray(np.asarray(gc_W, dtype=f))
    fcW = np.ascontiguousarray(np.asarray(fc_W, dtype=f))
    fcWb = np.ascontiguousarray(
        np.concatenate([fcW, np.asarray(fc_b, dtype=f).reshape(1, NCLASS)], axis=0))
    gcb1 = np.ascontiguousarray(
        np.concatenate([np.asarray(gc_b, dtype=f).reshape(NHID, 1),
                        np.ones((1, 1), dtype=f)], axis=0))
    return [{"xP": xP, "adjT": adjT[c], "gcW": gcW, "fcW": fcW,
             "fcWb": fcWb, "gcb1": gcb1} for c in range(NCORES)]


def run_traced(x, adj, gc_W, gc_b, fc_W, fc_b, trace=False, **kw):
    """Run on the 8 NeuronCores; returns (out [N, NCLASS] f32, BassKernelResults)."""
    from concourse.bass_utils import run_bass_kernel_spmd

    nc = _get_nc()
    in_maps = _prep_in_maps(x, adj, gc_W, gc_b, fc_W, fc_b)
    res = run_bass_kernel_spmd(nc, in_maps, list(range(NCORES)), trace=trace, **kw)
    outT = np.concatenate([res.results[c]["outT"] for c in range(NCORES)], axis=1)
    out = np.ascontiguousarray(outT.T).astype(np.float32, copy=False)
    return out, res


def kernel(x, adj, gc_W, gc_b, fc_W, fc_b):
    out, _ = run_traced(x, adj, gc_W, gc_b, fc_W, fc_b, trace=False)
    return out
